# revision 31
# baseline (speedup 1.0000x reference)
"""HAN (2 GAT edge types + semantic attention) on 8 trn2 NeuronCores.

v2.4: edges sorted by dst, dst-range sharded across cores, 128-row dst
windows. bf16 data-parallel projections; compact per-node tables
[h d-major | al_src] (260 cols) AllGathered then repacked once into
768B-stride rows so batched dma_gather ops (int16 idx, <=1024 rows/op,
table shards <=32768 rows) can fetch 520B/row. al_dst lives in local
256B-stride rows written at projection time and is gathered per edge the
same way (8B/row). Per-edge softmax uses num/den accumulation; the
segment-sum is a one-hot matmul whose lane->dst one-hots are built on DVE
per window (pad lanes get dl=-1 -> zero column). The semantic-attention
stage (tanh(out@klw+klb) column sums, y=out@lw) is fused into the edge
loop; per-type y and the 2-way softmax combine in a small tail.
"""
import numpy as np

import concourse.bass as bass
import concourse.bacc as bacc
import concourse.mybir as mybir
import concourse.tile as tile
from concourse.bass_utils import run_bass_kernel_spmd
from concourse.masks import make_identity
from concourse.ap_utils import ap_is_contiguous
from concourse._compat import exact_div

H, D = 4, 64
C = H * D
N_CORES = 8
P = 128
CE = C + H          # 260: h | al_src
CN = C + 3 * H      # 268 proj news: h | al_src_sim | al_dst_posts | al_dst_sim
CY = C + 2          # 258: klw | lw fused rhs
CP = 384            # padded table row (768B stride)
SH = 32000          # gather-shard rows (int16 indices)
MAXNI = 1024        # dma_gather per-op index cap
NQ = 1              # swdge queues (Tile locks DMA sems to one queue)

fp32 = mybir.dt.float32
bf16 = mybir.dt.bfloat16
i16 = mybir.dt.int16
BF = mybir.dt.np(bf16)

TRACE = False
TRACE_KW = {}
LAST_RESULTS = None

_PERM_DM = np.array([(j % H) * D + j // H for j in range(C)], np.int64)
_QN = [0]


def _block_diag_att(att):  # [H, D] -> [C, H] (h-major rows)
    A = np.zeros((C, H), np.float32)
    for h in range(H):
        A[h * D:(h + 1) * D, h] = att[h]
    return A


def _wrap16(idx):
    """index list [n] -> dma_gather idxs layout [128, n/16] i16 (wrapped over
    16 partitions, replicated to the 8 q7 cores); n must be 16-multiple."""
    n = len(idx)
    a = np.ascontiguousarray(np.asarray(idx, np.int16).reshape(n // 16, 16).T)
    return np.tile(a, (8, 1))


def _dma_gather_raw(nc, out_ap, in_ap, idxs_ap, num_idxs, elem_size, elem_step):
    """bass dma_gather minus the elem_size%256 assert (ucode only requires the
    row STRIDE to be 256B-aligned) plus queue round-robin."""
    g = nc.gpsimd
    assert num_idxs <= MAXNI
    assert idxs_ap.dtype == mybir.dt.int16
    assert in_ap.dtype == out_ap.dtype
    assert ap_is_contiguous(out_ap.ap[1:])
    assert ap_is_contiguous(idxs_ap.ap[1:])
    assert in_ap.ap[-1][1] == out_ap.ap[-1][1] == elem_size
    assert out_ap.ap[0][1] * out_ap.ap[1][1] == ((num_idxs + 127) // 128) * 128
    assert in_ap.ap[0][0] == elem_step
    stride_bytes_256 = exact_div(elem_step * mybir.dt.size(in_ap.dtype), 256)
    assert stride_bytes_256 < 256
    _QN[0] = 0
    _in_ap = g.lower_ap_dma(in_ap, for_custom_bir_dma=True)
    _idxs_ap = g.lower_ap(idxs_ap)
    _out_ap = g.lower_ap(out_ap)
    return g.add_instruction(
        mybir.InstDMAGatherAnt(
            name=nc.get_next_instruction_name(),
            ins=[*_in_ap, _idxs_ap, g.lower_val_access(g.to_reg(num_idxs))],
            outs=[_out_ap],
            transpose=False,
            num_idxs=num_idxs,
            elem_size=elem_size,
            stride_bytes_256=stride_bytes_256,
            gen_mode=0,
            single_packet=True,
            queue_num=_QN[0],
            sbuf_tokens_per_rank=0,
            sbuf_free_dim_per_rank=0,
            sbuf_free_dim_pad_per_rank=0,
            sbuf_byte_offset=0,
        )
    )


def _pack_type(src, dst, dst_lo, n_win, n_src):
    """Edges of one type for one core -> per (window, shard) edge lists.
    Returns (n_sh, T_sh, lists[w][s] = (src_local, dloc_lane))."""
    n_sh = -(-n_src // SH)
    dloc = dst - dst_lo
    win = dloc // P
    shard = src // SH
    lists = [[None] * n_sh for _ in range(n_win)]
    t_max = 1
    for w in range(n_win):
        mw = win == w
        for s in range(n_sh):
            m = mw & (shard == s)
            lists[w][s] = (src[m] - s * SH, dloc[m] - w * P)
            t_max = max(t_max, -(-int(m.sum()) // P))
    return n_sh, t_max, lists


def _meta_type(lists, n_win, n_sh, T):
    """-> gi [n_win, n_sh, 128, T*8] i16 (h-row gather idx, wrap16),
    adi [n_win, 128, n_sh*T*8] i16 (al_dst idx, tile order (s, t)),
    dl [n_win, P, n_sh*T] bf16 (lane dst row, -1 pad)."""
    NI = T * P
    gi = np.zeros((n_win, n_sh, 128, NI // 16), np.int16)
    adi = np.zeros((n_win, 128, n_sh * NI // 16), np.int16)
    dl = np.full((n_win, P, n_sh * T), -1.0, np.float32)
    for w in range(n_win):
        ad_all = np.zeros(n_sh * NI, np.int64)
        for s in range(n_sh):
            srcl, dloc = lists[w][s]
            n = len(srcl)
            sl = np.zeros(NI, np.int64)
            sl[:n] = srcl
            gi[w, s] = _wrap16(sl)
            ad_all[s * NI:s * NI + n] = dloc + w * P
            da = np.full(NI, -1.0, np.float32)
            da[:n] = dloc
            dl[w, :, s * T:(s + 1) * T] = da.reshape(T, P).T
        adi[w] = _wrap16(ad_all)
    return gi, adi, dl.astype(BF)


def build_program(n_news, n_user, f_in, ns, us, n_win, shp, tshp, shs, tshs):
    nc = bacc.Bacc(None, target_bir_lowering=False, num_swdge_queues=NQ)
    KC = f_in // P
    nsp = n_win * P
    usp = -(-us // P) * P
    n_ut = usp // P
    NIP = tshp * P
    NIS = tshs * P

    xuT = nc.declare_dram_parameter("xuT", [f_in + 1, usp], bf16, isOutput=False)
    xnT = nc.declare_dram_parameter("xnT", [f_in + 1, nsp], bf16, isOutput=False)
    wu = nc.declare_dram_parameter("wu", [f_in + 1, CE], bf16, isOutput=False)
    wn = nc.declare_dram_parameter("wn", [f_in + 1, CN], bf16, isOutput=False)
    gi_p = nc.declare_dram_parameter("gi_p", [n_win, shp, 128, NIP // 16], i16, isOutput=False)
    adi_p = nc.declare_dram_parameter("adi_p", [n_win, 128, shp * NIP // 16], i16, isOutput=False)
    dl_p = nc.declare_dram_parameter("dl_p", [n_win, P, shp * tshp], bf16, isOutput=False)
    gi_s = nc.declare_dram_parameter("gi_s", [n_win, shs, 128, NIS // 16], i16, isOutput=False)
    adi_s = nc.declare_dram_parameter("adi_s", [n_win, 128, shs * NIS // 16], i16, isOutput=False)
    dl_s = nc.declare_dram_parameter("dl_s", [n_win, P, shs * tshs], bf16, isOutput=False)
    klwx = nc.declare_dram_parameter("klwx", [C, CY], bf16, isOutput=False)
    klbx = nc.declare_dram_parameter("klbx", [1, CY], bf16, isOutput=False)
    qv = nc.declare_dram_parameter("qv", [1, C], fp32, isOutput=False)
    lb = nc.declare_dram_parameter("lb", [1, 2], fp32, isOutput=False)
    out_fin = nc.declare_dram_parameter("out", [nsp, 2], fp32, isOutput=True)

    ag_u = nc.dram_tensor("ag_u", [us, CE], bf16)
    ag_n = nc.dram_tensor("ag_n", [ns, CE], bf16)
    tbl_cu = nc.dram_tensor("tbl_cu", [n_user, CE], bf16, addr_space="Shared")
    tbl_cn = nc.dram_tensor("tbl_cn", [n_news, CE], bf16, addr_space="Shared")
    tbl_u = nc.dram_tensor("tbl_u", [n_user, CP], bf16)   # 768B-stride repack
    tbl_n = nc.dram_tensor("tbl_n", [n_news, CP], bf16)
    aldp = nc.dram_tensor("aldp", [nsp, 128], bf16)       # local, 256B stride
    alds = nc.dram_tensor("alds", [nsp, 128], bf16)
    y_p = nc.dram_tensor("y_p", [nsp, 2], fp32)
    y_s = nc.dram_tensor("y_s", [nsp, 2], fp32)
    s_in = nc.dram_tensor("s_in", [1, 4], fp32)
    s_out = nc.dram_tensor("s_out", [1, 4], fp32, addr_space="Shared")
    attn_d = nc.dram_tensor("attn_d", [1, 2], fp32)

    rg = [list(range(N_CORES))]

    with tile.TileContext(nc) as tc:
        with (
            tc.tile_pool(name="const", bufs=1) as cp,
            tc.tile_pool(name="wpool", bufs=1) as wp,
            tc.tile_pool(name="sb", bufs=3) as sb,
            tc.tile_pool(name="gat", bufs=3) as gp,
            tc.tile_pool(name="sem", bufs=2) as sm,
            tc.tile_pool(name="ps", bufs=2, space="PSUM") as ps,
            tc.tile_pool(name="pkf", bufs=2, space="PSUM") as pkf,
            tc.tile_pool(name="ptp", bufs=2, space="PSUM") as ptp,
            tc.tile_pool(name="pcs", bufs=2, space="PSUM") as pcs,
        ):
            iota_i = cp.tile([P, P], mybir.dt.int32)
            nc.gpsimd.iota(iota_i[:], pattern=[[1, P]], base=0, channel_multiplier=0)
            iota_b = cp.tile([P, P], bf16)
            nc.vector.tensor_copy(out=iota_b[:], in_=iota_i[:])
            ident_b = cp.tile([P, P], bf16)
            make_identity(nc, ident_b[:])
            ones1 = cp.tile([1, P], bf16)
            nc.vector.memset(ones1[:], 1.0)
            onesP = cp.tile([P, 1], bf16)
            nc.vector.memset(onesP[:], 1.0)

            wu_t = [wp.tile([P, CE], bf16, tag=f"wu{c}", name=f"wu{c}") for c in range(KC)]
            wub = wp.tile([1, CE], bf16, tag="wub")
            wn_t = [wp.tile([P, CN], bf16, tag=f"wn{c}", name=f"wn{c}") for c in range(KC)]
            wnb = wp.tile([1, CN], bf16, tag="wnb")
            for c in range(KC):
                nc.sync.dma_start(out=wu_t[c][:], in_=wu[c * P:(c + 1) * P, :])
                nc.sync.dma_start(out=wn_t[c][:], in_=wn[c * P:(c + 1) * P, :])
            nc.sync.dma_start(out=wub[:], in_=wu[f_in:f_in + 1, :])
            nc.sync.dma_start(out=wnb[:], in_=wn[f_in:f_in + 1, :])
            klwx_t = [wp.tile([P, CY], bf16, tag=f"klwx{c}", name=f"klwx{c}")
                      for c in range(2)]
            for c in range(2):
                nc.sync.dma_start(out=klwx_t[c][:], in_=klwx[c * P:(c + 1) * P, :])
            klbx_t = wp.tile([1, CY], bf16, tag="klbx")
            nc.sync.dma_start(out=klbx_t[:], in_=klbx[:])

            def project(n_tiles, xT, w_tiles, w_bias, width, sink):
                for nt in range(n_tiles):
                    pr = ps.tile([P, CN], fp32, space="PSUM", tag="mm")
                    for c in range(KC):
                        xt = sb.tile([P, P], bf16, tag="xT")
                        nc.sync.dma_start(
                            out=xt[:], in_=xT[c * P:(c + 1) * P, nt * P:(nt + 1) * P])
                        nc.tensor.matmul(out=pr[:, :width], lhsT=xt[:],
                                         rhs=w_tiles[c][:], start=(c == 0), stop=False)
                    o1 = sb.tile([1, P], bf16, tag="xT1")
                    nc.sync.dma_start(out=o1[:], in_=xT[f_in:f_in + 1, nt * P:(nt + 1) * P])
                    nc.tensor.matmul(out=pr[:, :width], lhsT=o1[:], rhs=w_bias[:],
                                     start=False, stop=True)
                    hp = sb.tile([P, CN], bf16, tag="hproj")
                    nc.scalar.copy(out=hp[:, :width], in_=pr[:, :width])
                    sink(nt, hp)

            def sink_news(nt, hp):
                r = min(P, ns - nt * P)
                nc.sync.dma_start(out=ag_n[nt * P:nt * P + r, :], in_=hp[:r, :CE])
                nc.sync.dma_start(out=aldp[nt * P:(nt + 1) * P, :H],
                                  in_=hp[:, C + H:C + 2 * H])
                nc.sync.dma_start(out=alds[nt * P:(nt + 1) * P, :H],
                                  in_=hp[:, C + 2 * H:C + 3 * H])

            def sink_user(nt, hp):
                r = min(P, us - nt * P)
                nc.sync.dma_start(out=ag_u[nt * P:nt * P + r, :], in_=hp[:r, :CE])

            def repack(src_t, dst_t, rows):
                nchunk = 8
                step = -(-rows // nchunk)
                for i in range(0, rows, step):
                    r = min(step, rows - i)
                    nc.sync.dma_start(out=dst_t[i:i + r, :CE], in_=src_t[i:i + r, :])

            project(n_win, xnT, wn_t, wnb, CN, sink_news)
            nc.gpsimd.collective_compute(
                "AllGather", mybir.AluOpType.bypass, replica_groups=rg,
                ins=[ag_n[:]], outs=[tbl_cn[:]])
            repack(tbl_cn, tbl_n, n_news)

            project(n_ut, xuT, wu_t, wub, CE, sink_user)
            nc.gpsimd.collective_compute(
                "AllGather", mybir.AluOpType.bypass, replica_groups=rg,
                ins=[ag_u[:]], outs=[tbl_cu[:]])
            repack(tbl_cu, tbl_u, n_user)

            # ---- edge phase ----
            def edge_type(n_sh, T, gi, adi, dl, tbl, ald_t, n_tbl, y_dram, cs_tag):
                NI = T * P
                TW = n_sh * T          # tiles per window
                csum = pcs.tile([1, C], fp32, space="PSUM", tag="csum")
                for w in range(n_win):
                    # gathers: h|als rows per shard into one window tile
                    g_win = gp.tile([P, TW * CE], bf16, tag="g")
                    gv = g_win[:].rearrange("p (t c) -> p t c", c=CE)
                    for s in range(n_sh):
                        gix = gp.tile([P, NI // 16], i16, tag="gix")
                        nc.sync.dma_start(out=gix[:], in_=gi[w, s])
                        lo = s * SH
                        hi = min(lo + 32768, n_tbl)
                        for c0 in range(0, T, 8):
                            cn_ = min(8, T - c0)
                            _dma_gather_raw(
                                nc, gv[:, s * T + c0:s * T + c0 + cn_, :],
                                tbl[lo:hi, :CE], gix[:, c0 * 8:(c0 + cn_) * 8],
                                cn_ * P, CE, CP)
                    adix = gp.tile([P, TW * P // 16], i16, tag="adix")
                    nc.sync.dma_start(out=adix[:], in_=adi[w])
                    ad_win = gp.tile([P, TW * H], bf16, tag="ad")
                    av = ad_win[:].rearrange("p (t h) -> p t h", h=H)
                    for c0 in range(0, TW, 8):
                        cn_ = min(8, TW - c0)
                        _dma_gather_raw(nc, av[:, c0:c0 + cn_, :], ald_t[:, :H],
                                        adix[:, c0 * 8:(c0 + cn_) * 8],
                                        cn_ * P, H, 128)
                    # one-hot scatter matrices (lane -> dst row; pad dl=-1 -> 0)
                    dlt = gp.tile([P, TW], bf16, tag="dlt")
                    nc.sync.dma_start(out=dlt[:], in_=dl[w])
                    s_win = gp.tile([P, TW * P], bf16, tag="s")
                    nc.vector.tensor_tensor(
                        out=s_win[:].rearrange("p (t q) -> p t q", q=P),
                        in0=dlt[:].rearrange("p (t x) -> p t x", x=1
                                             ).to_broadcast([P, TW, P]),
                        in1=iota_b[:].rearrange("p (x q) -> p x q", x=1
                                                ).to_broadcast([P, TW, P]),
                        op=mybir.AluOpType.is_equal)
                    # alpha, e = exp(leaky_relu(alpha)) batched over the window
                    al = gp.tile([P, TW * H], bf16, tag="al")
                    nc.vector.tensor_tensor(
                        out=al[:].rearrange("p (t h) -> p t h", h=H),
                        in0=gv[:, :, C:CE], in1=av[:],
                        op=mybir.AluOpType.add)
                    e1 = gp.tile([P, TW * H], bf16, tag="e1")
                    nc.scalar.activation(out=e1[:], in_=al[:],
                                         func=mybir.ActivationFunctionType.Exp)
                    e2 = gp.tile([P, TW * H], bf16, tag="e2")
                    nc.scalar.activation(out=e2[:], in_=al[:],
                                         func=mybir.ActivationFunctionType.Exp,
                                         scale=0.2)
                    ew = gp.tile([P, TW * H], bf16, tag="ew")
                    nc.vector.tensor_tensor(out=ew[:], in0=e1[:], in1=e2[:],
                                            op=mybir.AluOpType.max)
                    # m = h * e (d-major 2x) | e appended for the denominator
                    m_win = gp.tile([P, TW * CE], bf16, tag="m")
                    mv = m_win[:].rearrange("p (t c) -> p t c", c=CE)
                    nc.vector.tensor_tensor(
                        out=mv[:, :, :C].rearrange("p t (d h) -> p t d h", h=H),
                        in0=gv[:, :, :C].rearrange("p t (d h) -> p t d h", h=H),
                        in1=ew[:].rearrange("p (t x h) -> p t x h", x=1, h=H
                                            ).to_broadcast([P, TW, D, H]),
                        op=mybir.AluOpType.mult)
                    nc.vector.tensor_copy(
                        out=mv[:, :, C:CE],
                        in_=ew[:].rearrange("p (t h) -> p t h", h=H))
                    acc = ps.tile([P, CE], fp32, space="PSUM", tag="mm")
                    for t in range(TW):
                        nc.tensor.matmul(out=acc[:],
                                         lhsT=s_win[:, t * P:(t + 1) * P],
                                         rhs=m_win[:, t * CE:(t + 1) * CE],
                                         start=(t == 0), stop=(t == TW - 1))
                    # out = relu(num/den), then fused semantic attention
                    den = sm.tile([P, H], fp32, tag="den")
                    nc.vector.tensor_scalar(out=den[:], in0=acc[:, C:CE],
                                            scalar1=1e-8, scalar2=None,
                                            op0=mybir.AluOpType.max)
                    rec = sm.tile([P, H], fp32, tag="rec")
                    nc.vector.reciprocal(out=rec[:], in_=den[:])
                    ot = sm.tile([P, C], bf16, tag="ot")
                    nc.vector.tensor_tensor(
                        out=ot[:].rearrange("p (d h) -> p d h", h=H),
                        in0=acc[:, :C].rearrange("p (d h) -> p d h", h=H),
                        in1=rec[:].rearrange("p (x h) -> p x h", x=1
                                             ).to_broadcast([P, D, H]),
                        op=mybir.AluOpType.mult)
                    nc.vector.tensor_scalar(out=ot[:], in0=ot[:], scalar1=0.0,
                                            scalar2=None, op0=mybir.AluOpType.max)
                    otr = []
                    for c in range(2):
                        tp = ptp.tile([P, P], bf16, space="PSUM", tag="tp")
                        nc.tensor.transpose(out=tp[:], in_=ot[:, c * P:(c + 1) * P],
                                            identity=ident_b[:])
                        tr = sm.tile([P, P], bf16, tag="tr")
                        nc.vector.tensor_copy(out=tr[:], in_=tp[:])
                        otr.append(tr)
                    kf = pkf.tile([P, CY], fp32, space="PSUM", tag="kf")
                    for c in range(2):
                        nc.tensor.matmul(out=kf[:], lhsT=otr[c][:], rhs=klwx_t[c][:],
                                         start=(c == 0), stop=False)
                    nc.tensor.matmul(out=kf[:], lhsT=ones1[:], rhs=klbx_t[:],
                                     start=False, stop=True)
                    th = sm.tile([P, C], bf16, tag="th")
                    nc.scalar.activation(out=th[:], in_=kf[:, :C],
                                         func=mybir.ActivationFunctionType.Tanh)
                    nc.tensor.matmul(out=csum[:], lhsT=onesP[:], rhs=th[:],
                                     start=(w == 0), stop=(w == n_win - 1))
                    ysb = sm.tile([P, 2], fp32, tag="ysb")
                    nc.vector.tensor_copy(out=ysb[:], in_=kf[:, C:CY])
                    nc.sync.dma_start(out=y_dram[w * P:(w + 1) * P, :], in_=ysb[:])
                cs = sm.tile([1, C], fp32, tag="cs")
                q_t = sm.tile([1, C], fp32, tag="qt")
                nc.sync.dma_start(out=q_t[:], in_=qv[:])
                nc.vector.tensor_tensor(out=cs[:], in0=csum[:], in1=q_t[:],
                                        op=mybir.AluOpType.mult)
                sv = sm.tile([1, 1], fp32, tag="sv")
                nc.vector.reduce_sum(out=sv[:], in_=cs[:], axis=mybir.AxisListType.X)
                si = sm.tile([1, 4], fp32, tag=f"si{cs_tag}")
                nc.vector.memset(si[:], 0.0)
                nc.vector.tensor_copy(out=si[:, cs_tag:cs_tag + 1], in_=sv[:])
                return si

            siS = edge_type(shs, tshs, gi_s, adi_s, dl_s, tbl_n, alds,
                            n_news, y_s, 1)
            siP = edge_type(shp, tshp, gi_p, adi_p, dl_p, tbl_u, aldp,
                            n_user, y_p, 0)
            sisum = sm.tile([1, 4], fp32, tag="sisum")
            nc.vector.tensor_tensor(out=sisum[:], in0=siP[:], in1=siS[:],
                                    op=mybir.AluOpType.add)
            nc.sync.dma_start(out=s_in[:], in_=sisum[:])
            nc.gpsimd.collective_compute(
                "AllReduce", mybir.AluOpType.add, replica_groups=rg,
                ins=[s_in[:]], outs=[s_out[:]])

            sc = sm.tile([1, 2], fp32, tag="sc")
            nc.sync.dma_start(out=sc[:], in_=s_out[:1, :2])
            nc.vector.tensor_scalar(out=sc[:], in0=sc[:], scalar1=1.0 / n_news,
                                    scalar2=None, op0=mybir.AluOpType.mult)
            mx = sm.tile([1, 1], fp32, tag="mx")
            nc.vector.reduce_max(out=mx[:], in_=sc[:], axis=mybir.AxisListType.X)
            ex = sm.tile([1, 2], fp32, tag="ex")
            nc.vector.tensor_scalar(out=ex[:], in0=sc[:], scalar1=mx[:, :1],
                                    scalar2=None, op0=mybir.AluOpType.subtract)
            nc.scalar.activation(out=ex[:], in_=ex[:],
                                 func=mybir.ActivationFunctionType.Exp)
            smv = sm.tile([1, 1], fp32, tag="smv")
            nc.vector.reduce_sum(out=smv[:], in_=ex[:], axis=mybir.AxisListType.X)
            rc = sm.tile([1, 1], fp32, tag="rc")
            nc.vector.reciprocal(out=rc[:], in_=smv[:])
            at = sm.tile([1, 2], fp32, tag="at")
            nc.vector.tensor_scalar(out=at[:], in0=ex[:], scalar1=rc[:, :1],
                                    scalar2=None, op0=mybir.AluOpType.mult)
            nc.sync.dma_start(out=attn_d[:], in_=at[:])
            atb = sm.tile([P, 2], fp32, tag="atb")
            nc.sync.dma_start(out=atb[:], in_=attn_d[:].to_broadcast((P, 2)))
            lbb = sm.tile([P, 2], fp32, tag="lbb")
            nc.sync.dma_start(out=lbb[:], in_=lb[:].to_broadcast((P, 2)))
            for nt in range(n_win):
                ypt = sb.tile([P, 2], fp32, tag="ypt")
                nc.sync.dma_start(out=ypt[:], in_=y_p[nt * P:(nt + 1) * P, :])
                yst = sb.tile([P, 2], fp32, tag="yst")
                nc.sync.dma_start(out=yst[:], in_=y_s[nt * P:(nt + 1) * P, :])
                f1 = sb.tile([P, 2], fp32, tag="f1")
                nc.vector.tensor_scalar(out=f1[:], in0=ypt[:], scalar1=atb[:, 0:1],
                                        scalar2=None, op0=mybir.AluOpType.mult)
                f2 = sb.tile([P, 2], fp32, tag="f2")
                nc.vector.tensor_scalar(out=f2[:], in0=yst[:], scalar1=atb[:, 1:2],
                                        scalar2=None, op0=mybir.AluOpType.mult)
                nc.vector.tensor_tensor(out=f1[:], in0=f1[:], in1=f2[:],
                                        op=mybir.AluOpType.add)
                nc.vector.tensor_tensor(out=f1[:], in0=f1[:], in1=lbb[:],
                                        op=mybir.AluOpType.add)
                nc.sync.dma_start(out=out_fin[nt * P:(nt + 1) * P, :], in_=f1[:])
    nc.compile()
    return nc


def build_nc_for_inputs(**inputs):
    x_news = np.asarray(inputs["x_news"], np.float32)
    x_user = np.asarray(inputs["x_user"], np.float32)
    posts_src = np.asarray(inputs["posts_src"]).astype(np.int64)
    posts_dst = np.asarray(inputs["posts_dst"]).astype(np.int64)
    sim_src = np.asarray(inputs["sim_src"]).astype(np.int64)
    sim_dst = np.asarray(inputs["sim_dst"]).astype(np.int64)

    n_news, f_in = x_news.shape
    n_user = x_user.shape[0]
    ns = n_news // N_CORES
    us = n_user // N_CORES
    n_win = -(-ns // P)
    nsp = n_win * P
    usp = -(-us // P) * P

    Wn = np.asarray(inputs["proj_news_w"], np.float32)
    bn = np.asarray(inputs["proj_news_b"], np.float32)
    Wu = np.asarray(inputs["proj_user_w"], np.float32)
    bu = np.asarray(inputs["proj_user_b"], np.float32)
    A_sp = _block_diag_att(np.asarray(inputs["att_src_posts"], np.float32))
    A_dp = _block_diag_att(np.asarray(inputs["att_dst_posts"], np.float32))
    A_ss = _block_diag_att(np.asarray(inputs["att_src_sim"], np.float32))
    A_ds = _block_diag_att(np.asarray(inputs["att_dst_sim"], np.float32))
    wu_ext = np.concatenate([Wu[:, _PERM_DM], Wu @ A_sp], 1)
    bu_ext = np.concatenate([bu[_PERM_DM], bu @ A_sp])
    wn_ext = np.concatenate([Wn[:, _PERM_DM], Wn @ A_ss, Wn @ A_dp, Wn @ A_ds], 1)
    bn_ext = np.concatenate([bn[_PERM_DM], bn @ A_ss, bn @ A_dp, bn @ A_ds])
    wu_full = np.concatenate([wu_ext, bu_ext[None]], 0).astype(BF)
    wn_full = np.concatenate([wn_ext, bn_ext[None]], 0).astype(BF)

    klw = np.asarray(inputs["k_lin_w"], np.float32)[_PERM_DM, :]
    lw = np.asarray(inputs["lin_w"], np.float32)[_PERM_DM, :]
    klwx = np.concatenate([klw, lw], 1).astype(BF)
    klbx = np.concatenate([np.asarray(inputs["k_lin_b"], np.float32),
                           np.zeros(2, np.float32)])[None].astype(BF)

    pp = np.argsort(posts_dst, kind="stable")
    ps_src, ps_dst = posts_src[pp], posts_dst[pp]
    sp = np.argsort(sim_dst, kind="stable")
    sm_src, sm_dst = sim_src[sp], sim_dst[sp]
    pb = np.searchsorted(ps_dst, np.arange(N_CORES + 1) * ns)
    sb_ = np.searchsorted(sm_dst, np.arange(N_CORES + 1) * ns)

    packs = []
    T_P = T_S = 1
    shp = shs = 1
    for k in range(N_CORES):
        nshp, tp, lp = _pack_type(ps_src[pb[k]:pb[k + 1]], ps_dst[pb[k]:pb[k + 1]],
                                  k * ns, n_win, n_user)
        nshs, tsv, ls = _pack_type(sm_src[sb_[k]:sb_[k + 1]], sm_dst[sb_[k]:sb_[k + 1]],
                                   k * ns, n_win, n_news)
        packs.append((lp, ls))
        T_P, T_S = max(T_P, tp), max(T_S, tsv)
        shp, shs = nshp, nshs

    xuT_all = np.concatenate([x_user.T, np.ones((1, n_user), np.float32)], 0)
    xnT_all = np.concatenate([x_news.T, np.ones((1, n_news), np.float32)], 0)
    in_maps = []
    for k in range(N_CORES):
        lp, ls = packs[k]
        gi_p, adi_p, dl_p = _meta_type(lp, n_win, shp, T_P)
        gi_s, adi_s, dl_s = _meta_type(ls, n_win, shs, T_S)
        xuT_k = np.zeros((f_in + 1, usp), np.float32)
        xuT_k[:, :us] = xuT_all[:, k * us:(k + 1) * us]
        xnT_k = np.zeros((f_in + 1, nsp), np.float32)
        xnT_k[:, :ns] = xnT_all[:, k * ns:(k + 1) * ns]
        in_maps.append({
            "xuT": xuT_k.astype(BF), "xnT": xnT_k.astype(BF),
            "wu": wu_full, "wn": wn_full,
            "gi_p": gi_p, "adi_p": adi_p, "dl_p": dl_p,
            "gi_s": gi_s, "adi_s": adi_s, "dl_s": dl_s,
            "klwx": klwx, "klbx": klbx,
            "qv": np.asarray(inputs["q"], np.float32)[None],
            "lb": np.asarray(inputs["lin_b"], np.float32)[None],
        })

    nc = build_program(n_news, n_user, f_in, ns, us, n_win, shp, T_P, shs, T_S)
    return nc, in_maps


def kernel(**inputs):
    n_news = np.asarray(inputs["x_news"]).shape[0]
    ns = n_news // N_CORES
    nc, in_maps = build_nc_for_inputs(**inputs)
    r = run_bass_kernel_spmd(nc, in_maps, list(range(N_CORES)),
                             trace=TRACE, **TRACE_KW)
    globals()["LAST_RESULTS"] = r
    res = r.results
    out = np.empty((n_news, 2), np.float32)
    for k in range(N_CORES):
        out[k * ns:(k + 1) * ns] = res[k]["out"][:ns]
    return out
